# revision 22
# baseline (speedup 1.0000x reference)
"""Trainium2 Bass kernel for GQA attention with RoPE (dense transformer).

Problem: B=2, S=2048, H=2048, 16 query heads / 4 KV heads, head_dim 128,
causal flash-style attention, fused QKV + o_proj.

Sharding (8 cores): (batch, head-group) grid. Core c handles batch c//4 and
head group c%4 (4 query heads + their shared KV head). o_proj is computed as
per-group partials reduced on host (tensor-parallel o_proj input split).

V4: fp8e4m3 + DoubleRow (0.565 cyc/row) for every 256-deep contraction:
  - q/k/v projections: x and weights fp8, contraction 2048 = 8 DR pairs.
    Weights are scaled x16 host-side (else sigma~0.002 lands in fp8
    subnormals); q/k carry x16 into bf16 RoPE outputs and the scores
    matmul, compensated by exp(x/256 - 1) on ACT (the -1 is a global
    shift, softmax-invariant, and buys fp8 overflow margin: max logit
    measured 5.49, e^4.49=89 << 448). v's x16 is removed at eviction.
  - attention over k: exp tiles written fp8 (pairs [128,2,512] for full
    k-tiles, singles for causal-diagonal partial tiles), v fp8;
    attn@v and the ones-rowsum run DR over pairs, plain fp8 on singles.
  - o_proj: contraction 512 = 2 DR pairs; ofl fp8 (DVE writes bf16 -
    DVE fp8 stores are broken - GpSimd converts), wo fp8 x16
    compensated by ones=16 (p_sum scales, so 1/sum normalization
    divides the 16 back out).
Scores stay bf16 (contraction 128: DoubleRow not applicable).
PSUM fp32 throughout; softmax normalization in fp32 on DVE.

Pipeline: projections front-loaded, attention chunks woven in as inputs
appear, o_proj spread through the ACT(exp)-bound tail so the PE stays
busy and HAM never re-throttles. PSUM banks: A0,A1 (proj sweeps),
S0,S1 (scores), P0 (ones), O0 (attn@v), C0,C1 (o_proj / v-transpose).
"""
import math

import numpy as np
import ml_dtypes

import concourse.bass as bass
import concourse.mybir as mybir
import concourse.tile as tile
from concourse import bacc
from concourse.bass_utils import run_bass_kernel_spmd
from concourse.masks import make_identity

B, S, H = 2, 2048, 2048
NH, KVH, HD = 16, 4, 128
G = 4                 # head groups (= KVH); grid = G x B = 8 cores
GQ = NH // KVH        # query heads per group
QD = GQ * HD          # per-core q dim (512)
KP = H // 256         # DR contraction pairs for projections (8)
TC = 4                # token chunks of 512
TT = S // 128         # 128-token tiles (16)

F32 = mybir.dt.float32
BF16 = mybir.dt.bfloat16
FP8 = mybir.dt.float8e4
AF = mybir.ActivationFunctionType
DR = mybir.MatmulPerfMode.DoubleRow

PROJ_FP8 = False       # x/wq/wk/wv fp8 + DR projections
EX_FP8 = True          # exp tiles fp8: ones-rowsum runs DoubleRow; v stays
                       # bf16 (v-fp8 alone busts the error budget; mixed
                       # bf16-stationary x fp8-moving matmul is exact on HW)
OPROJ_FP8 = False      # ofl/wo fp8 + DR o_proj
OUT16 = True           # bf16 output partials (halves output DMA)

WSCALE = 16.0         # fp8 weight rescale (kills subnormals)

_NC = None


def _emit(nc):
    in8 = FP8 if PROJ_FP8 else BF16
    o8 = FP8 if OPROJ_FP8 else BF16
    e8 = FP8 if EX_FP8 else BF16
    out_dt = BF16 if OUT16 else F32
    xT = nc.dram_tensor("xT", [H, S], in8, kind="ExternalInput").ap()
    wqT = nc.dram_tensor("wqT", [H, QD], in8, kind="ExternalInput").ap()
    wkT = nc.dram_tensor("wkT", [H, HD], in8, kind="ExternalInput").ap()
    wvT = nc.dram_tensor("wvT", [H, HD], in8, kind="ExternalInput").ap()
    woT = nc.dram_tensor("woT", [QD, H], o8, kind="ExternalInput").ap()
    cosT = nc.dram_tensor("cosT", [HD, S], F32, kind="ExternalInput").ap()
    sinS = nc.dram_tensor("sinS", [HD, S], F32, kind="ExternalInput").ap()
    bqkv = nc.dram_tensor("bqkv", [128, 7], F32, kind="ExternalInput").ap()
    onesd = nc.dram_tensor("onesd", [256, 128], e8, kind="ExternalInput").ap()
    outp = nc.dram_tensor("outp", [S, H], out_dt,
                      kind="ExternalOutput").ap()

    if PROJ_FP8:
        xT4 = xT.rearrange("(kp s p) t -> p kp s t", p=128, s=2)
        wqT4 = wqT.rearrange("(kp s p) m -> p kp s m", p=128, s=2)
        wkT4 = wkT.rearrange("(kp s p) m -> p kp s m", p=128, s=2)
        wvT4 = wvT.rearrange("(kp s p) m -> p kp s m", p=128, s=2)
    else:
        xT4 = xT.rearrange("(ko p) t -> p ko t", p=128)
        wqT4 = wqT.rearrange("(ko p) m -> p ko m", p=128)
        wkT4 = wkT.rearrange("(ko p) m -> p ko m", p=128)
        wvT4 = wvT.rearrange("(ko p) m -> p ko m", p=128)
    if OPROJ_FP8:
        woT4 = woT.rearrange("(hp s p) o -> p hp s o", p=128, s=2)
    else:
        woT4 = woT.rearrange("(ic p) o -> p ic o", p=128)

    with tile.TileContext(nc) as tc:
        with (
            tc.tile_pool(name="persist", bufs=1) as pp,
            tc.tile_pool(name="qfp", bufs=4) as pqf,
            tc.tile_pool(name="cd", bufs=1) as pd,
            tc.tile_pool(name="expp", bufs=1) as pe,
            tc.tile_pool(name="psum8", bufs=1, space="PSUM") as ps8,
            tc.tile_pool(name="projw", bufs=1) as pw,
            tc.tile_pool(name="projx", bufs=1) as px,
            tc.tile_pool(name="rope", bufs=1) as pr,
            tc.tile_pool(name="outs", bufs=1) as po,
        ):
            # persistent per-chunk K/V
            kf = [pp.tile([128, 512], BF16, name=f"kf{t}") for t in range(TC)]
            v_sb = [pp.tile([128, 4, HD], BF16, name=f"vsb{t}")
                    for t in range(TC)]
            ofl = pd.tile([128, GQ, S], o8)       # normalized attn outT

            # ---- constants ----
            bias_sb = pp.tile([128, 7], F32)
            nc.gpsimd.dma_start(bias_sb[:, :], bqkv)
            ident = pp.tile([128, 128], BF16)
            make_identity(nc, ident[:, :])
            ones_dr = pp.tile([128, 2, 128], e8)
            nc.gpsimd.dma_start(
                ones_dr[:, :, :],
                onesd.rearrange("(s p) m -> p s m", p=128))

            # weights / tables
            if PROJ_FP8:
                wq_sb = pw.tile([128, KP, 2, QD], FP8)
                wk_sb = pw.tile([128, KP, 2, HD], FP8)
                wv_sb = pw.tile([128, KP, 2, HD], FP8)
            else:
                wq_sb = pw.tile([128, 2 * KP, QD], BF16)
                wk_sb = pw.tile([128, 2 * KP, HD], BF16)
                wv_sb = pw.tile([128, 2 * KP, HD], BF16)
            if OPROJ_FP8:
                wo_sb = pw.tile([128, 2, 2, H], FP8)
            else:
                wo_sb = pw.tile([128, GQ, H], BF16)
            cos_sb = pw.tile([128, S], F32)
            sin_sb = pw.tile([128, S], F32)
            if PROJ_FP8:
                xcs = [px.tile([128, KP, 2, 512], FP8, tag=f"xc{t % 2}",
                               name=f"xc_{t}", bufs=1) for t in range(TC)]
            else:
                xcs = [px.tile([128, 2 * KP, 512], BF16, tag=f"xc{t % 2}",
                               name=f"xc_{t}", bufs=1) for t in range(TC)]

            qf_tiles = [None] * TC

            def warmup_spin(n):
                for i in range(n):
                    w = ps8.tile([128, 128], F32, tag="C0",
                                 name=f"spin_{i}")
                    nc.tensor.matmul(w[:, :], ident[:, :], ident[:, :],
                                     start=True, stop=True)

            def jspan(qc, j):
                if j < 4 * qc:
                    q0, n = 512 * qc, 512
                else:
                    q0 = 128 * j
                    n = 512 * (qc + 1) - q0
                return q0, n, q0 - 512 * qc

            exp_scale = (1.0 / (WSCALE * WSCALE)) if PROJ_FP8 else 1.0
            exp_bias = bias_sb[:, 6:7]

            def attention_h(qc, h):
                """flash attention for (q-chunk qc, head h).

                k-tiles 0..4qc-1 are full (512-wide q span) and processed as
                DR pairs; 4qc..4qc+3 are causal-diagonal partials, processed
                as plain fp8 matmuls with the triangle zeroed post-exp.
                """
                qs = slice(512 * qc, 512 * qc + 512)
                qf_t = qf_tiles[qc]
                nj = 4 * qc + 4
                pairs = []      # [128, 2, 512] fp8
                singles = []    # (j, [128, 512] fp8)
                for j in range(nj):
                    q0, n, off = jspan(qc, j)
                    ql = q0 - 512 * qc
                    ps = ps8.tile([128, 512], F32, tag=f"S{j % 2}",
                                  name=f"ps_{h}_{qc}_{j}")
                    nc.tensor.matmul(
                        ps[:, 0:n], kf[j // 4][:, 128 * (j % 4):
                                               128 * (j % 4) + 128],
                        qf_t[:, h, ql:ql + n], start=True, stop=True)
                    if EX_FP8 and j < 4 * qc:
                        if j % 2 == 0:
                            ex2 = pe.tile([128, 2, 512], e8, tag="E2",
                                          bufs=8, name=f"ex2_{h}_{qc}_{j}")
                            pairs.append(ex2)
                        nc.scalar.activation(pairs[-1][:, j % 2, :],
                                             ps[:, :], AF.Exp,
                                             bias=exp_bias, scale=exp_scale)
                    else:
                        ex = pe.tile([128, 512], e8, tag="E1",
                                     bufs=(6 if EX_FP8 else 16),
                                     name=f"ex_{h}_{qc}_{j}")
                        nc.scalar.activation(ex[:, 0:n], ps[:, 0:n], AF.Exp,
                                             bias=exp_bias, scale=exp_scale)
                        if j >= 4 * qc:
                            # zero the strictly-lower (q < k) triangle
                            nc.gpsimd.affine_select(
                                out=ex[:, 0:128], in_=ex[:, 0:128],
                                compare_op=mybir.AluOpType.is_ge, fill=0.0,
                                base=0, pattern=[[1, 128]],
                                channel_multiplier=-1)
                        singles.append((j, ex))

                p_sum = ps8.tile([128, 512], F32, tag="P0",
                                 name=f"psum_{h}_{qc}")
                nmm = len(pairs) + len(singles)
                i = 0
                for ex2 in pairs:
                    nc.tensor.matmul(p_sum[:, :], ones_dr[:, :, :],
                                     ex2[:, :, :], perf_mode=DR,
                                     start=(i == 0), stop=(i == nmm - 1))
                    i += 1
                for j, ex in singles:
                    q0, n, off = jspan(qc, j)
                    nc.tensor.matmul(p_sum[:, off:off + n],
                                     ones_dr[:, 0, :], ex[:, 0:n],
                                     start=(i == 0), stop=(i == nmm - 1))
                    i += 1
                bc = pr.tile([128, 512], F32, tag="bc", bufs=2,
                             name=f"bc_{h}_{qc}")
                nc.vector.reciprocal_approx_fast(bc[:, :], p_sum[:, :])

                p_o = ps8.tile([128, 512], F32, tag="O0",
                               name=f"po_{h}_{qc}")
                nav = 2 * len(pairs) + len(singles)
                i = 0
                for jp, ex2 in enumerate(pairs):
                    for s2 in range(2):
                        j = 2 * jp + s2
                        nc.tensor.matmul(p_o[:, :],
                                         v_sb[j // 4][:, j % 4, :],
                                         ex2[:, s2, :],
                                         start=(i == 0), stop=(i == nav - 1))
                        i += 1
                for j, ex in singles:
                    q0, n, off = jspan(qc, j)
                    nc.tensor.matmul(p_o[:, off:off + n],
                                     v_sb[j // 4][:, j % 4, :], ex[:, 0:n],
                                     start=(i == 0), stop=(i == nav - 1))
                    i += 1
                if OPROJ_FP8:
                    o16 = pr.tile([128, 512], BF16, tag="o16", bufs=2,
                                  name=f"o16_{h}_{qc}")
                    nc.vector.tensor_mul(o16[:, :], p_o[:, :], bc[:, :])
                    nc.gpsimd.tensor_copy(ofl[:, h, qs], o16[:, :])
                else:
                    nc.vector.tensor_mul(ofl[:, h, qs], p_o[:, :], bc[:, :])

            def rope_evict(src_ps, bcol, dst, t):
                """ACT evict (bias add) then RoPE on DVE; dst is bf16."""
                ts = slice(512 * t, 512 * t + 512)
                raw = pr.tile([128, 512], F32, tag="raw", bufs=2,
                              name=f"raw_{t}_{bcol}")
                nc.scalar.activation(raw[:, :], src_ps, AF.Identity,
                                     bias=bias_sb[:, bcol:bcol + 1])
                rot = pr.tile([128, 512], F32, tag="rot", bufs=2,
                              name=f"rot_{t}_{bcol}")
                nc.vector.tensor_copy(rot[0:64, :], raw[64:128, :])
                nc.vector.tensor_copy(rot[64:128, :], raw[0:64, :])
                t1 = pr.tile([128, 512], F32, tag="t1", bufs=2,
                             name=f"t1_{t}_{bcol}")
                nc.vector.tensor_mul(t1[:, :], rot[:, :], sin_sb[:, ts])
                t2 = pr.tile([128, 512], F32, tag="t2", bufs=2,
                             name=f"t2_{t}_{bcol}")
                nc.vector.tensor_mul(t2[:, :], raw[:, :], cos_sb[:, ts])
                nc.vector.tensor_add(dst, t1[:, :], t2[:, :])

            def o_proj_chunk(qc):
                """o_proj partials for token chunk qc -> DRAM."""
                for tt in range(4 * qc, 4 * qc + 4):
                    tsl = slice(128 * tt, 128 * tt + 128)
                    stage = po.tile([128, 4, 512], out_dt, tag="st", bufs=2,
                                    name=f"st_{tt}")
                    for oc in range(4):
                        osl = slice(512 * oc, 512 * oc + 512)
                        pf = ps8.tile([128, 512], F32, tag=f"C{oc % 2}",
                                      name=f"pf_{tt}_{oc}")
                        if OPROJ_FP8:
                            for hp in range(2):
                                nc.tensor.matmul(
                                    pf[:, :],
                                    ofl[:, 2 * hp:2 * hp + 2, tsl],
                                    wo_sb[:, hp, :, osl], perf_mode=DR,
                                    start=(hp == 0), stop=(hp == 1))
                        else:
                            for ic in range(GQ):
                                nc.tensor.matmul(
                                    pf[:, :], ofl[:, ic, tsl],
                                    wo_sb[:, ic, osl],
                                    start=(ic == 0), stop=(ic == GQ - 1))
                        if oc % 2 == 0:
                            nc.vector.tensor_copy(stage[:, oc, :], pf[:, :])
                        else:
                            nc.scalar.copy(stage[:, oc, :], pf[:, :])
                    nc.gpsimd.dma_start(outp[tsl, :],
                                      stage.rearrange("p a b -> p (a b)"))

            def proj_mm(out_ps, w_sb, msl, t, last_q=False):
                """q/k/v projection accumulation sweep into one PSUM bank."""
                if PROJ_FP8:
                    for kp in range(KP):
                        nc.tensor.matmul(
                            out_ps[:, :], w_sb[:, kp, :, msl],
                            xcs[t][:, kp, :, :], perf_mode=DR,
                            start=(kp == 0), stop=(kp == KP - 1))
                else:
                    for ko in range(2 * KP):
                        nc.tensor.matmul(
                            out_ps[:, :], w_sb[:, ko, msl],
                            xcs[t][:, ko, :], start=(ko == 0),
                            stop=(ko == 2 * KP - 1))

            def proj_chunk(t):
                ts = slice(512 * t, 512 * t + 512)
                # --- DMA prefetches for chunk t ---
                if t == 0:
                    warmup_spin(64)
                    if PROJ_FP8:
                        for g4 in range(4):
                            nc.sync.dma_start(
                                wq_sb[:, 2 * g4:2 * g4 + 2, :, :],
                                wqT4[:, 2 * g4:2 * g4 + 2, :, :])
                            nc.sync.dma_start(
                                xcs[t][:, 2 * g4:2 * g4 + 2, :, :],
                                xT4[:, 2 * g4:2 * g4 + 2, :, ts])
                        nc.sync.dma_start(wk_sb[:, :, :, :], wkT4)
                        nc.sync.dma_start(wv_sb[:, :, :, :], wvT4)
                    else:
                        for g4 in range(4):
                            nc.sync.dma_start(
                                wq_sb[:, 4 * g4:4 * g4 + 4, :],
                                wqT4[:, 4 * g4:4 * g4 + 4, :])
                            nc.sync.dma_start(
                                xcs[t][:, 4 * g4:4 * g4 + 4, :],
                                xT4[:, 4 * g4:4 * g4 + 4, ts])
                        nc.sync.dma_start(wk_sb[:, :, :], wkT4)
                        nc.sync.dma_start(wv_sb[:, :, :], wvT4)
                elif PROJ_FP8:
                    nc.sync.dma_start(xcs[t][:, 0:4, :, :],
                                      xT4[:, 0:4, :, ts])
                    nc.sync.dma_start(xcs[t][:, 4:8, :, :],
                                      xT4[:, 4:8, :, ts])
                else:
                    nc.sync.dma_start(xcs[t][:, 0:8, :],
                                      xT4[:, 0:8, ts])
                    nc.sync.dma_start(xcs[t][:, 8:16, :],
                                      xT4[:, 8:16, ts])
                nc.sync.dma_start(cos_sb[:, ts], cosT[:, ts])
                nc.sync.dma_start(sin_sb[:, ts], sinS[:, ts])
                if t == 1:
                    if OPROJ_FP8:
                        nc.sync.dma_start(wo_sb[:, :, :, :], woT4)
                    else:
                        nc.sync.dma_start(wo_sb[:, :, :], woT4)

                qf_t = pqf.tile([128, GQ, 512], BF16, tag="qf",
                                name=f"qf_{t}")
                qf_tiles[t] = qf_t
                # --- sweep 1: q heads 0,1 ---
                pq0 = ps8.tile([128, 512], F32, tag="A0", name=f"pq0_{t}")
                pq1 = ps8.tile([128, 512], F32, tag="A1", name=f"pq1_{t}")
                proj_mm(pq0, wq_sb, slice(0, 128), t)
                proj_mm(pq1, wq_sb, slice(128, 256), t)
                rope_evict(pq0[:, :], 0, qf_t[:, 0, :], t)
                rope_evict(pq1[:, :], 1, qf_t[:, 1, :], t)
                # --- sweep 2: q heads 2,3 ---
                pq2 = ps8.tile([128, 512], F32, tag="A0", name=f"pq2_{t}")
                pq3 = ps8.tile([128, 512], F32, tag="A1", name=f"pq3_{t}")
                proj_mm(pq2, wq_sb, slice(256, 384), t)
                proj_mm(pq3, wq_sb, slice(384, 512), t)
                rope_evict(pq2[:, :], 2, qf_t[:, 2, :], t)
                rope_evict(pq3[:, :], 3, qf_t[:, 3, :], t)
                # --- sweep 3: k + v ---
                pk = ps8.tile([128, 512], F32, tag="A0", name=f"pk_{t}")
                pv = ps8.tile([128, 512], F32, tag="A1", name=f"pv_{t}")
                proj_mm(pk, wk_sb, slice(0, HD), t)
                proj_mm(pv, wv_sb, slice(0, HD), t)
                rope_evict(pk[:, :], 4, kf[t][:, :], t)
                # v: evict with bias (scale drops the x16), transpose,
                # convert to fp8 on the PSUM->SBUF copy
                vT_t = pr.tile([128, 512], BF16, tag="vT", bufs=2,
                               name=f"vT_{t}")
                nc.scalar.activation(vT_t[:, :], pv[:, :], AF.Identity,
                                     bias=bias_sb[:, 5:6],
                                     scale=(1.0 / WSCALE if PROJ_FP8
                                            else 1.0))
                for st4 in range(4):
                    ptr = ps8.tile([128, 128], BF16, tag=f"C{st4 % 2}",
                                   name=f"ptr_{t}_{st4}")
                    nc.tensor.transpose(
                        ptr[:, :], vT_t[:, 128 * st4:128 * st4 + 128],
                        ident[:, :])
                    nc.scalar.copy(v_sb[t][:, st4, :], ptr[:, :])

            # Front-load projections; weave attention chunks in as soon as
            # their inputs exist; spread o_proj through the ACT-bound tail so
            # the PE never idles long enough for HAM to re-throttle.
            proj_chunk(0)
            proj_chunk(1)
            for h in range(GQ):
                attention_h(0, h)
            proj_chunk(2)
            for h in range(GQ):
                attention_h(1, h)
            proj_chunk(3)
            attention_h(2, 0)
            attention_h(2, 1)
            o_proj_chunk(0)
            attention_h(2, 2)
            attention_h(2, 3)
            attention_h(3, 0)
            o_proj_chunk(1)
            attention_h(3, 1)
            attention_h(3, 2)
            o_proj_chunk(2)
            attention_h(3, 3)
            o_proj_chunk(3)


def _build():
    global _NC
    if _NC is None:
        nc = bacc.Bacc("TRN2", target_bir_lowering=False, debug=False,
                       num_devices=8)
        _emit(nc)
        nc.compile()
        _NC = nc
    return _NC


def _prep_inputs(x, wq, bq, wk, bk, wv, bv, wo, bo, cos, sin):
    """Host-side shard + layout prep. Core c = (g, b): g = c % 4, b = c // 4."""
    inv_sqrt_d = 1.0 / math.sqrt(HD)
    f32 = np.float32
    bf16 = ml_dtypes.bfloat16
    fp8 = ml_dtypes.float8_e4m3fn
    in8 = fp8 if PROJ_FP8 else bf16
    o8 = fp8 if OPROJ_FP8 else bf16
    e8 = fp8 if EX_FP8 else bf16
    ws = WSCALE if PROJ_FP8 else 1.0
    wos = WSCALE if OPROJ_FP8 else 1.0
    # ones stays 1.0: scaling it would push ofl = o/16 into fp8 subnormals;
    # the wo x16 is divided back out on the host after the partial gather.
    ones_val = 1.0

    cosT = np.ascontiguousarray(cos.T.astype(f32))
    sinS = np.ascontiguousarray(sin.T.astype(f32))
    sinS[0:HD // 2] *= -1.0

    xTb = [np.ascontiguousarray(x[b].T.astype(in8)) for b in range(B)]

    in_maps = []
    for c in range(8):
        g, b = c % G, c // G
        wq_s = wq[QD * g:QD * (g + 1), :] * (inv_sqrt_d * ws)
        bq_s = bq[QD * g:QD * (g + 1)] * (inv_sqrt_d * ws)
        wk_s = wk[HD * g:HD * (g + 1), :] * ws
        bk_s = bk[HD * g:HD * (g + 1)] * ws
        wv_s = wv[HD * g:HD * (g + 1), :] * ws
        bv_s = bv[HD * g:HD * (g + 1)]
        bias = np.zeros((128, 7), f32)
        bias[:, 6] = -1.0 if EX_FP8 else 0.0
        bias[:, 0:4] = bq_s.reshape(GQ, HD).T
        bias[:, 4] = bk_s
        bias[:, 5] = bv_s
        in_maps.append({
            "xT": xTb[b],
            "wqT": np.ascontiguousarray(wq_s.T.astype(in8)),
            "wkT": np.ascontiguousarray(wk_s.T.astype(in8)),
            "wvT": np.ascontiguousarray(wv_s.T.astype(in8)),
            "woT": np.ascontiguousarray((wo[:, QD * g:QD * (g + 1)] * wos).T
                                        .astype(o8)),
            "cosT": cosT,
            "sinS": sinS,
            "bqkv": bias,
            "onesd": np.full((256, 128), ones_val, e8),
        })
    return in_maps


def run(inputs, trace=False):
    """Returns (full_output, BassKernelResults)."""
    inputs = {k: np.asarray(v) for k, v in inputs.items()}
    nc = _build()
    in_maps = _prep_inputs(**inputs)
    res = run_bass_kernel_spmd(nc, in_maps, core_ids=list(range(8)),
                               trace=trace)
    bo = inputs["bo"].astype(np.float64)
    oscale = 1.0 / WSCALE if OPROJ_FP8 else 1.0
    out = np.empty((B, S, H), np.float32)
    for b in range(B):
        acc = np.zeros((S, H), np.float64)
        for g in range(G):
            acc += res.results[G * b + g]["outp"].astype(np.float64)
        out[b] = (acc * oscale + bo).astype(np.float32)
    return out, res


def kernel(**inputs):
    return run(inputs, trace=False)[0]


# revision 23
# speedup vs baseline: 1.1914x; 1.1914x over previous
"""Trainium2 Bass kernel for GQA attention with RoPE (dense transformer).

Problem: B=2, S=2048, H=2048, 16 query heads / 4 KV heads, head_dim 128,
causal flash-style attention, fused QKV + o_proj.

Sharding (8 cores): (batch, head-group) grid. Core c handles batch c//4 and
head group c%4 (4 query heads + their shared KV head). o_proj is computed as
per-group partials reduced on host (tensor-parallel o_proj input split).

V4: fp8e4m3 + DoubleRow (0.565 cyc/row) for every 256-deep contraction:
  - q/k/v projections: x and weights fp8, contraction 2048 = 8 DR pairs.
    Weights are scaled x16 host-side (else sigma~0.002 lands in fp8
    subnormals); q/k carry x16 into bf16 RoPE outputs and the scores
    matmul, compensated by exp(x/256 - 1) on ACT (the -1 is a global
    shift, softmax-invariant, and buys fp8 overflow margin: max logit
    measured 5.49, e^4.49=89 << 448). v's x16 is removed at eviction.
  - attention over k: exp tiles written fp8 (pairs [128,2,512] for full
    k-tiles, singles for causal-diagonal partial tiles), v fp8;
    attn@v and the ones-rowsum run DR over pairs, plain fp8 on singles.
  - o_proj: contraction 512 = 2 DR pairs; ofl fp8 (DVE writes bf16 -
    DVE fp8 stores are broken - GpSimd converts), wo fp8 x16
    compensated by ones=16 (p_sum scales, so 1/sum normalization
    divides the 16 back out).
Scores stay bf16 (contraction 128: DoubleRow not applicable).
PSUM fp32 throughout; softmax normalization in fp32 on DVE.

Pipeline: projections front-loaded, attention chunks woven in as inputs
appear, o_proj spread through the ACT(exp)-bound tail so the PE stays
busy and HAM never re-throttles. PSUM banks: A0,A1 (proj sweeps),
S0,S1 (scores), P0 (ones), O0 (attn@v), C0,C1 (o_proj / v-transpose).
"""
import math

import numpy as np
import ml_dtypes

import concourse.bass as bass
import concourse.mybir as mybir
import concourse.tile as tile
from concourse import bacc
from concourse.bass_utils import run_bass_kernel_spmd
from concourse.masks import make_identity

B, S, H = 2, 2048, 2048
NH, KVH, HD = 16, 4, 128
G = 4                 # head groups (= KVH); grid = G x B = 8 cores
GQ = NH // KVH        # query heads per group
QD = GQ * HD          # per-core q dim (512)
KP = H // 256         # DR contraction pairs for projections (8)
TC = 4                # token chunks of 512
TT = S // 128         # 128-token tiles (16)

F32 = mybir.dt.float32
BF16 = mybir.dt.bfloat16
FP8 = mybir.dt.float8e4
AF = mybir.ActivationFunctionType
DR = mybir.MatmulPerfMode.DoubleRow

PROJ_FP8 = False       # x/wq/wk/wv fp8 + DR projections
EX_FP8 = True          # exp tiles fp8: ones-rowsum runs DoubleRow; v stays
                       # bf16 (v-fp8 alone busts the error budget; mixed
                       # bf16-stationary x fp8-moving matmul is exact on HW)
OPROJ_FP8 = False      # ofl/wo fp8 + DR o_proj
OUT16 = True           # bf16 output partials (halves output DMA)

WSCALE = 16.0         # fp8 weight rescale (kills subnormals)

_NC = None


def _emit(nc):
    in8 = FP8 if PROJ_FP8 else BF16
    o8 = FP8 if OPROJ_FP8 else BF16
    e8 = FP8 if EX_FP8 else BF16
    out_dt = BF16 if OUT16 else F32
    xT = nc.dram_tensor("xT", [H, S], in8, kind="ExternalInput").ap()
    wqT = nc.dram_tensor("wqT", [H, QD], in8, kind="ExternalInput").ap()
    wkT = nc.dram_tensor("wkT", [H, HD], in8, kind="ExternalInput").ap()
    wvT = nc.dram_tensor("wvT", [H, HD], in8, kind="ExternalInput").ap()
    woT = nc.dram_tensor("woT", [QD, H], o8, kind="ExternalInput").ap()
    cosT = nc.dram_tensor("cosT", [HD, S], F32, kind="ExternalInput").ap()
    sinS = nc.dram_tensor("sinS", [HD, S], F32, kind="ExternalInput").ap()
    bqkv = nc.dram_tensor("bqkv", [128, 7], F32, kind="ExternalInput").ap()
    onesd = nc.dram_tensor("onesd", [256, 128], e8, kind="ExternalInput").ap()
    outp = nc.dram_tensor("outp", [S, H], out_dt,
                      kind="ExternalOutput").ap()

    if PROJ_FP8:
        xT4 = xT.rearrange("(kp s p) t -> p kp s t", p=128, s=2)
        wqT4 = wqT.rearrange("(kp s p) m -> p kp s m", p=128, s=2)
        wkT4 = wkT.rearrange("(kp s p) m -> p kp s m", p=128, s=2)
        wvT4 = wvT.rearrange("(kp s p) m -> p kp s m", p=128, s=2)
    else:
        xT4 = xT.rearrange("(ko p) t -> p ko t", p=128)
        wqT4 = wqT.rearrange("(ko p) m -> p ko m", p=128)
        wkT4 = wkT.rearrange("(ko p) m -> p ko m", p=128)
        wvT4 = wvT.rearrange("(ko p) m -> p ko m", p=128)
    if OPROJ_FP8:
        woT4 = woT.rearrange("(hp s p) o -> p hp s o", p=128, s=2)
    else:
        woT4 = woT.rearrange("(ic p) o -> p ic o", p=128)

    with tile.TileContext(nc) as tc:
        with (
            tc.tile_pool(name="persist", bufs=1) as pp,
            tc.tile_pool(name="qfp", bufs=4) as pqf,
            tc.tile_pool(name="cd", bufs=1) as pd,
            tc.tile_pool(name="expp", bufs=1) as pe,
            tc.tile_pool(name="psum8", bufs=1, space="PSUM") as ps8,
            tc.tile_pool(name="projw", bufs=1) as pw,
            tc.tile_pool(name="projx", bufs=1) as px,
            tc.tile_pool(name="rope", bufs=1) as pr,
            tc.tile_pool(name="outs", bufs=1) as po,
        ):
            # persistent per-chunk K/V
            kf = [pp.tile([128, 512], BF16, name=f"kf{t}") for t in range(TC)]
            v_sb = [pp.tile([128, 4, HD], BF16, name=f"vsb{t}")
                    for t in range(TC)]
            ofl = pd.tile([128, GQ, S], o8)       # normalized attn outT

            # ---- constants ----
            bias_sb = pp.tile([128, 7], F32)
            nc.gpsimd.dma_start(bias_sb[:, :], bqkv)
            ident = pp.tile([128, 128], BF16)
            make_identity(nc, ident[:, :])
            ones_dr = pp.tile([128, 2, 128], e8)
            nc.gpsimd.dma_start(
                ones_dr[:, :, :],
                onesd.rearrange("(s p) m -> p s m", p=128))

            # weights / tables
            if PROJ_FP8:
                wq_sb = pw.tile([128, KP, 2, QD], FP8)
                wk_sb = pw.tile([128, KP, 2, HD], FP8)
                wv_sb = pw.tile([128, KP, 2, HD], FP8)
            else:
                wq_sb = pw.tile([128, 2 * KP, QD], BF16)
                wk_sb = pw.tile([128, 2 * KP, HD], BF16)
                wv_sb = pw.tile([128, 2 * KP, HD], BF16)
            if OPROJ_FP8:
                wo_sb = pw.tile([128, 2, 2, H], FP8)
            else:
                wo_sb = pw.tile([128, GQ, H], BF16)
            cos_sb = pw.tile([128, S], F32)
            sin_sb = pw.tile([128, S], F32)
            if PROJ_FP8:
                xcs = [px.tile([128, KP, 2, 512], FP8, tag=f"xc{t % 2}",
                               name=f"xc_{t}", bufs=1) for t in range(TC)]
            else:
                xcs = [px.tile([128, 2 * KP, 512], BF16, tag=f"xc{t % 2}",
                               name=f"xc_{t}", bufs=1) for t in range(TC)]

            qf_tiles = [None] * TC

            def jspan(qc, j):
                if j < 4 * qc:
                    q0, n = 512 * qc, 512
                else:
                    q0 = 128 * j
                    n = 512 * (qc + 1) - q0
                return q0, n, q0 - 512 * qc

            exp_scale = (1.0 / (WSCALE * WSCALE)) if PROJ_FP8 else 1.0
            exp_bias = bias_sb[:, 6:7]

            def attention_h(qc, h):
                """flash attention for (q-chunk qc, head h).

                k-tiles 0..4qc-1 are full (512-wide q span) and processed as
                DR pairs; 4qc..4qc+3 are causal-diagonal partials, processed
                as plain fp8 matmuls with the triangle zeroed post-exp.
                """
                qs = slice(512 * qc, 512 * qc + 512)
                qf_t = qf_tiles[qc]
                nj = 4 * qc + 4
                pairs = []      # [128, 2, 512] fp8
                singles = []    # (j, [128, 512] fp8)
                for j in range(nj):
                    q0, n, off = jspan(qc, j)
                    ql = q0 - 512 * qc
                    ps = ps8.tile([128, 512], F32, tag=f"S{j % 2}",
                                  name=f"ps_{h}_{qc}_{j}")
                    nc.tensor.matmul(
                        ps[:, 0:n], kf[j // 4][:, 128 * (j % 4):
                                               128 * (j % 4) + 128],
                        qf_t[:, h, ql:ql + n], start=True, stop=True)
                    if EX_FP8 and j < 4 * qc:
                        if j % 2 == 0:
                            ex2 = pe.tile([128, 2, 512], e8, tag="E2",
                                          bufs=8, name=f"ex2_{h}_{qc}_{j}")
                            pairs.append(ex2)
                        nc.scalar.activation(pairs[-1][:, j % 2, :],
                                             ps[:, :], AF.Exp,
                                             bias=exp_bias, scale=exp_scale)
                    else:
                        ex = pe.tile([128, 512], e8, tag="E1",
                                     bufs=(6 if EX_FP8 else 16),
                                     name=f"ex_{h}_{qc}_{j}")
                        nc.scalar.activation(ex[:, 0:n], ps[:, 0:n], AF.Exp,
                                             bias=exp_bias, scale=exp_scale)
                        if j >= 4 * qc:
                            # zero the strictly-lower (q < k) triangle
                            nc.gpsimd.affine_select(
                                out=ex[:, 0:128], in_=ex[:, 0:128],
                                compare_op=mybir.AluOpType.is_ge, fill=0.0,
                                base=0, pattern=[[1, 128]],
                                channel_multiplier=-1)
                        singles.append((j, ex))

                p_sum = ps8.tile([128, 512], F32, tag="P0",
                                 name=f"psum_{h}_{qc}")
                nmm = len(pairs) + len(singles)
                i = 0
                for ex2 in pairs:
                    nc.tensor.matmul(p_sum[:, :], ones_dr[:, :, :],
                                     ex2[:, :, :], perf_mode=DR,
                                     start=(i == 0), stop=(i == nmm - 1))
                    i += 1
                for j, ex in singles:
                    q0, n, off = jspan(qc, j)
                    nc.tensor.matmul(p_sum[:, off:off + n],
                                     ones_dr[:, 0, :], ex[:, 0:n],
                                     start=(i == 0), stop=(i == nmm - 1))
                    i += 1
                bc = pr.tile([128, 512], F32, tag="bc", bufs=2,
                             name=f"bc_{h}_{qc}")
                nc.vector.reciprocal_approx_fast(bc[:, :], p_sum[:, :])

                p_o = ps8.tile([128, 512], F32, tag="O0",
                               name=f"po_{h}_{qc}")
                nav = 2 * len(pairs) + len(singles)
                i = 0
                for jp, ex2 in enumerate(pairs):
                    for s2 in range(2):
                        j = 2 * jp + s2
                        nc.tensor.matmul(p_o[:, :],
                                         v_sb[j // 4][:, j % 4, :],
                                         ex2[:, s2, :],
                                         start=(i == 0), stop=(i == nav - 1))
                        i += 1
                for j, ex in singles:
                    q0, n, off = jspan(qc, j)
                    nc.tensor.matmul(p_o[:, off:off + n],
                                     v_sb[j // 4][:, j % 4, :], ex[:, 0:n],
                                     start=(i == 0), stop=(i == nav - 1))
                    i += 1
                if OPROJ_FP8:
                    o16 = pr.tile([128, 512], BF16, tag="o16", bufs=2,
                                  name=f"o16_{h}_{qc}")
                    nc.vector.tensor_mul(o16[:, :], p_o[:, :], bc[:, :])
                    nc.gpsimd.tensor_copy(ofl[:, h, qs], o16[:, :])
                else:
                    nc.vector.tensor_mul(ofl[:, h, qs], p_o[:, :], bc[:, :])

            def rope_evict(src_ps, bcol, dst, t):
                """ACT evict (bias add) then RoPE on DVE; dst is bf16."""
                ts = slice(512 * t, 512 * t + 512)
                raw = pr.tile([128, 512], F32, tag="raw", bufs=2,
                              name=f"raw_{t}_{bcol}")
                nc.scalar.activation(raw[:, :], src_ps, AF.Identity,
                                     bias=bias_sb[:, bcol:bcol + 1])
                rot = pr.tile([128, 512], F32, tag="rot", bufs=2,
                              name=f"rot_{t}_{bcol}")
                nc.vector.tensor_copy(rot[0:64, :], raw[64:128, :])
                nc.vector.tensor_copy(rot[64:128, :], raw[0:64, :])
                t1 = pr.tile([128, 512], F32, tag="t1", bufs=2,
                             name=f"t1_{t}_{bcol}")
                nc.vector.tensor_mul(t1[:, :], rot[:, :], sin_sb[:, ts])
                t2 = pr.tile([128, 512], F32, tag="t2", bufs=2,
                             name=f"t2_{t}_{bcol}")
                nc.vector.tensor_mul(t2[:, :], raw[:, :], cos_sb[:, ts])
                nc.vector.tensor_add(dst, t1[:, :], t2[:, :])

            def o_proj_chunk(qc):
                """o_proj partials for token chunk qc -> DRAM."""
                for tt in range(4 * qc, 4 * qc + 4):
                    tsl = slice(128 * tt, 128 * tt + 128)
                    stage = po.tile([128, 4, 512], out_dt, tag="st", bufs=2,
                                    name=f"st_{tt}")
                    for oc in range(4):
                        osl = slice(512 * oc, 512 * oc + 512)
                        pf = ps8.tile([128, 512], F32, tag=f"C{oc % 2}",
                                      name=f"pf_{tt}_{oc}")
                        if OPROJ_FP8:
                            for hp in range(2):
                                nc.tensor.matmul(
                                    pf[:, :],
                                    ofl[:, 2 * hp:2 * hp + 2, tsl],
                                    wo_sb[:, hp, :, osl], perf_mode=DR,
                                    start=(hp == 0), stop=(hp == 1))
                        else:
                            for ic in range(GQ):
                                nc.tensor.matmul(
                                    pf[:, :], ofl[:, ic, tsl],
                                    wo_sb[:, ic, osl],
                                    start=(ic == 0), stop=(ic == GQ - 1))
                        if oc % 2 == 0:
                            nc.vector.tensor_copy(stage[:, oc, :], pf[:, :])
                        else:
                            nc.scalar.copy(stage[:, oc, :], pf[:, :])
                    nc.sync.dma_start(outp[tsl, :],
                                      stage.rearrange("p a b -> p (a b)"))

            def proj_mm(out_ps, w_sb, msl, t, last_q=False):
                """q/k/v projection accumulation sweep into one PSUM bank."""
                if PROJ_FP8:
                    for kp in range(KP):
                        nc.tensor.matmul(
                            out_ps[:, :], w_sb[:, kp, :, msl],
                            xcs[t][:, kp, :, :], perf_mode=DR,
                            start=(kp == 0), stop=(kp == KP - 1))
                else:
                    for ko in range(2 * KP):
                        nc.tensor.matmul(
                            out_ps[:, :], w_sb[:, ko, msl],
                            xcs[t][:, ko, :], start=(ko == 0),
                            stop=(ko == 2 * KP - 1))

            def proj_chunk(t):
                ts = slice(512 * t, 512 * t + 512)
                # --- DMA prefetches for chunk t ---
                if t == 0:
                    if PROJ_FP8:
                        nc.sync.dma_start(wq_sb[:, :, :, :], wqT4)
                        nc.sync.dma_start(wk_sb[:, :, :, :], wkT4)
                        nc.sync.dma_start(wv_sb[:, :, :, :], wvT4)
                    else:
                        nc.sync.dma_start(wq_sb[:, :, :], wqT4)
                        nc.sync.dma_start(wk_sb[:, :, :], wkT4)
                        nc.sync.dma_start(wv_sb[:, :, :], wvT4)
                if PROJ_FP8:
                    nc.sync.dma_start(xcs[t][:, 0:4, :, :],
                                      xT4[:, 0:4, :, ts])
                    nc.sync.dma_start(xcs[t][:, 4:8, :, :],
                                      xT4[:, 4:8, :, ts])
                else:
                    nc.sync.dma_start(xcs[t][:, 0:8, :],
                                      xT4[:, 0:8, ts])
                    nc.sync.dma_start(xcs[t][:, 8:16, :],
                                      xT4[:, 8:16, ts])
                nc.sync.dma_start(cos_sb[:, ts], cosT[:, ts])
                nc.sync.dma_start(sin_sb[:, ts], sinS[:, ts])
                if t == 1:
                    if OPROJ_FP8:
                        nc.sync.dma_start(wo_sb[:, :, :, :], woT4)
                    else:
                        nc.sync.dma_start(wo_sb[:, :, :], woT4)

                qf_t = pqf.tile([128, GQ, 512], BF16, tag="qf",
                                name=f"qf_{t}")
                qf_tiles[t] = qf_t
                # --- sweep 1: q heads 0,1 ---
                pq0 = ps8.tile([128, 512], F32, tag="A0", name=f"pq0_{t}")
                pq1 = ps8.tile([128, 512], F32, tag="A1", name=f"pq1_{t}")
                proj_mm(pq0, wq_sb, slice(0, 128), t)
                proj_mm(pq1, wq_sb, slice(128, 256), t)
                rope_evict(pq0[:, :], 0, qf_t[:, 0, :], t)
                rope_evict(pq1[:, :], 1, qf_t[:, 1, :], t)
                # --- sweep 2: q heads 2,3 ---
                pq2 = ps8.tile([128, 512], F32, tag="A0", name=f"pq2_{t}")
                pq3 = ps8.tile([128, 512], F32, tag="A1", name=f"pq3_{t}")
                proj_mm(pq2, wq_sb, slice(256, 384), t)
                proj_mm(pq3, wq_sb, slice(384, 512), t)
                rope_evict(pq2[:, :], 2, qf_t[:, 2, :], t)
                rope_evict(pq3[:, :], 3, qf_t[:, 3, :], t)
                # --- sweep 3: k + v ---
                pk = ps8.tile([128, 512], F32, tag="A0", name=f"pk_{t}")
                pv = ps8.tile([128, 512], F32, tag="A1", name=f"pv_{t}")
                proj_mm(pk, wk_sb, slice(0, HD), t)
                proj_mm(pv, wv_sb, slice(0, HD), t)
                rope_evict(pk[:, :], 4, kf[t][:, :], t)
                # v: evict with bias (scale drops the x16), transpose,
                # convert to fp8 on the PSUM->SBUF copy
                vT_t = pr.tile([128, 512], BF16, tag="vT", bufs=2,
                               name=f"vT_{t}")
                nc.scalar.activation(vT_t[:, :], pv[:, :], AF.Identity,
                                     bias=bias_sb[:, 5:6],
                                     scale=(1.0 / WSCALE if PROJ_FP8
                                            else 1.0))
                for st4 in range(4):
                    ptr = ps8.tile([128, 128], BF16, tag=f"C{st4 % 2}",
                                   name=f"ptr_{t}_{st4}")
                    nc.tensor.transpose(
                        ptr[:, :], vT_t[:, 128 * st4:128 * st4 + 128],
                        ident[:, :])
                    nc.scalar.copy(v_sb[t][:, st4, :], ptr[:, :])

            # Front-load projections; weave attention chunks in as soon as
            # their inputs exist; spread o_proj through the ACT-bound tail so
            # the PE never idles long enough for HAM to re-throttle.
            proj_chunk(0)
            proj_chunk(1)
            for h in range(GQ):
                attention_h(0, h)
            proj_chunk(2)
            for h in range(GQ):
                attention_h(1, h)
            proj_chunk(3)
            attention_h(2, 0)
            attention_h(2, 1)
            o_proj_chunk(0)
            attention_h(2, 2)
            attention_h(2, 3)
            attention_h(3, 0)
            o_proj_chunk(1)
            attention_h(3, 1)
            attention_h(3, 2)
            o_proj_chunk(2)
            attention_h(3, 3)
            o_proj_chunk(3)


def _build():
    global _NC
    if _NC is None:
        nc = bacc.Bacc("TRN2", target_bir_lowering=False, debug=False,
                       num_devices=8)
        _emit(nc)
        nc.compile()
        _NC = nc
    return _NC


def _prep_inputs(x, wq, bq, wk, bk, wv, bv, wo, bo, cos, sin):
    """Host-side shard + layout prep. Core c = (g, b): g = c % 4, b = c // 4."""
    inv_sqrt_d = 1.0 / math.sqrt(HD)
    f32 = np.float32
    bf16 = ml_dtypes.bfloat16
    fp8 = ml_dtypes.float8_e4m3fn
    in8 = fp8 if PROJ_FP8 else bf16
    o8 = fp8 if OPROJ_FP8 else bf16
    e8 = fp8 if EX_FP8 else bf16
    ws = WSCALE if PROJ_FP8 else 1.0
    wos = WSCALE if OPROJ_FP8 else 1.0
    # ones stays 1.0: scaling it would push ofl = o/16 into fp8 subnormals;
    # the wo x16 is divided back out on the host after the partial gather.
    ones_val = 1.0

    cosT = np.ascontiguousarray(cos.T.astype(f32))
    sinS = np.ascontiguousarray(sin.T.astype(f32))
    sinS[0:HD // 2] *= -1.0

    xTb = [np.ascontiguousarray(x[b].T.astype(in8)) for b in range(B)]

    in_maps = []
    for c in range(8):
        g, b = c % G, c // G
        wq_s = wq[QD * g:QD * (g + 1), :] * (inv_sqrt_d * ws)
        bq_s = bq[QD * g:QD * (g + 1)] * (inv_sqrt_d * ws)
        wk_s = wk[HD * g:HD * (g + 1), :] * ws
        bk_s = bk[HD * g:HD * (g + 1)] * ws
        wv_s = wv[HD * g:HD * (g + 1), :] * ws
        bv_s = bv[HD * g:HD * (g + 1)]
        bias = np.zeros((128, 7), f32)
        bias[:, 6] = -1.0 if EX_FP8 else 0.0
        bias[:, 0:4] = bq_s.reshape(GQ, HD).T
        bias[:, 4] = bk_s
        bias[:, 5] = bv_s
        in_maps.append({
            "xT": xTb[b],
            "wqT": np.ascontiguousarray(wq_s.T.astype(in8)),
            "wkT": np.ascontiguousarray(wk_s.T.astype(in8)),
            "wvT": np.ascontiguousarray(wv_s.T.astype(in8)),
            "woT": np.ascontiguousarray((wo[:, QD * g:QD * (g + 1)] * wos).T
                                        .astype(o8)),
            "cosT": cosT,
            "sinS": sinS,
            "bqkv": bias,
            "onesd": np.full((256, 128), ones_val, e8),
        })
    return in_maps


def run(inputs, trace=False):
    """Returns (full_output, BassKernelResults)."""
    inputs = {k: np.asarray(v) for k, v in inputs.items()}
    nc = _build()
    in_maps = _prep_inputs(**inputs)
    res = run_bass_kernel_spmd(nc, in_maps, core_ids=list(range(8)),
                               trace=trace)
    bo = inputs["bo"].astype(np.float64)
    oscale = 1.0 / WSCALE if OPROJ_FP8 else 1.0
    out = np.empty((B, S, H), np.float32)
    for b in range(B):
        acc = np.zeros((S, H), np.float64)
        for g in range(G):
            acc += res.results[G * b + g]["outp"].astype(np.float64)
        out[b] = (acc * oscale + bo).astype(np.float32)
    return out, res


def kernel(**inputs):
    return run(inputs, trace=False)[0]
